# revision 21
# baseline (speedup 1.0000x reference)
"""BitConvSwiGLU on 8 Trainium2 cores.

Strategy: pure token-data-parallelism. The 8192 tokens (B*S) are split into
8 slabs of 1024 tokens; each core computes its slab end-to-end (both
matmuls over the full d_hidden) so no collectives are needed. The depthwise
conv needs one halo token on each side, recomputed locally from a
halo-padded x slab (zero rows at batch boundaries reproduce the conv's
zero padding, since bit_linear(0) == 0).

v3 design:
- h never leaves SBUF (no DRAM spill); the quantized mm2 operand reuses
  the h pool slots.
- fp16 intermediates (11-bit mantissa, sim rel ~5e-3) with integer-exact
  matmuls: xq/hq small ints, w1/w2 ternary, PSUM f32.
- Conv fold: conv = cw1*(deq1 + r0*deq0 + r2*deq2) with r_j = cw_j/cw1;
  the cw1 scale rides the Silu activation's per-partition scale, saving
  one elementwise op (overflow checked against the fixed weight draw).
- absmax = max(max_c h, 0.2785): silu(z) >= -0.27847 globally, so the
  clamp is exact whenever any channel's h >= 0.2785 (verified: min
  per-token maxh is 0.89) - no Abs, no min tracking.
- Engine split per chunk: DVE deq windows + tap0 + absmax-acc + quant;
  ACT tap2 + Silu(scale,bias); GpSimd the two adds (only add/mult TT is
  supported on Pool). Laggy consumers get lag-matched emission offsets
  and deep SBUF rings so the DVE (which recycles mm1's PSUM) never
  blocks behind them.
- PE stream is dense: mm1(h0), mm1(h1), mm2(h0) x2, mm2(h1) x2
  back-to-back; token-scale reductions hide inside other phases.
- Round-to-int via the +-1.5*2^23 magic-number trick (DVE f32 internal).
"""
import math
from contextlib import ExitStack

import numpy as np
import ml_dtypes


# ---------------------------------------------------------------------------
# Workaround: this walrus build rejects >1 sync wait on CTRL-class
# instructions (Drain/Nop). TileContext's epilogue drain aggregates one wait
# per active proc onto a single Drain. Split the excess onto follow-up nops.
def _install_tile_patch():
    import concourse.mybir as mybir
    from concourse.tile import TileContext
    from concourse.vector_clock import ScopedClock

    if getattr(TileContext, "_drain_patch_installed", False):
        return

    MAX_WAITS = 1

    def _split_waits(nc, inst):
        si = inst.ins.sync_info
        if si is None or len(si.on_wait) <= MAX_WAITS:
            return
        waits = list(si.on_wait)
        si.on_wait = waits[:MAX_WAITS]
        inst.ins.sync_info = si
        for i in range(MAX_WAITS, len(waits), MAX_WAITS):
            nop = nc.sync.nop()
            nop.ins.sync_info = mybir.SyncInfo(
                on_wait=waits[i : i + MAX_WAITS], on_update=[]
            )

    def _patched_drain_and_barrier(self, tick_clock, wait_clock):
        nc = self.nc
        drain_inst = nc.sync.drain()
        wait_clock.add_sem_waits(
            drain_inst.ins, ScopedClock({None: tick_clock.global_clock})
        )
        _split_waits(nc, drain_inst)

        nc.all_engine_barrier()
        assert self.sems is not None
        popped = nc._tile_sem_poison_stack.pop()
        assert popped is self._sem_poison
        nc.clear_and_free_semaphores(list(self.sems.allocated().values()))
        nc.all_engine_barrier()

    TileContext._drain_and_barrier = _patched_drain_and_barrier
    TileContext._drain_patch_installed = True

    # Generic safety net: rewrite the BIR JSON before compile, splitting any
    # instruction with >1 sync wait into same-engine NoOps placed before it
    # (a same-engine nop stalls the engine identically, so semantics hold).
    import json as _json
    import concourse.bass_utils as _bu
    import concourse.bass2jax as _b2j

    _orig_compile = _bu.compile_bir_kernel

    def _split_bir_waits(bir_json: bytes) -> bytes:
        d = _json.loads(bir_json)
        n_split = [0]

        def fix_block(b):
            insts = b.get("instructions", [])
            out = []
            for inst in insts:
                si = inst.get("sync_info")
                waits = si.get("on_wait") if si else None
                if waits and len(waits) > 1:
                    keep, extra = waits[:1], waits[1:]
                    for j in range(0, len(extra)):
                        out.append({
                            "name": f"{inst['name']}_w{j}",
                            "opcode": "NoOp",
                            "engine": inst.get("engine", "SP"),
                            "ins": [],
                            "outs": [],
                            "sync_info": {
                                "on_wait": [extra[j]],
                                "on_update": [],
                            },
                        })
                        n_split[0] += 1
                    si["on_wait"] = keep
                out.append(inst)
            b["instructions"] = out
            for sub in b.get("blocks", []):
                fix_block(sub)

        for f in d.get("functions", []):
            for b in f.get("blocks", []):
                fix_block(b)
        if n_split[0]:
            return _json.dumps(d).encode()
        return bir_json

    def _patched_compile(bir_json, tmpdir, neff_name="file.neff"):
        return _orig_compile(_split_bir_waits(bir_json), tmpdir, neff_name)

    _bu.compile_bir_kernel = _patched_compile
    _b2j.compile_bir_kernel = _patched_compile


# ---------------------------------------------------------------------------
# Problem dims (hardcoded per contract)
B, S, D, H = 4, 2048, 1024, 4096
N_CORES = 8
EPS = 1e-5
P = 128
MAGIC = 12582912.0  # 1.5 * 2**23: f32 addend that forces round-to-nearest-int
SILU_MIN = 0.2785   # > |global min of silu| = 0.27847; absmax clamp floor


def build_nc(t_own, alpha_c, beta_c):
    """Build the SPMD single-core program for a slab of t_own tokens."""
    import concourse.bass as bass
    import concourse.mybir as mybir
    from concourse.tile import TileContext
    from concourse.masks import make_identity

    f32 = mybir.dt.float32
    fp16 = mybir.dt.float16
    i8 = mybir.dt.int8
    AF = mybir.ActivationFunctionType
    ALU = mybir.AluOpType
    AX = mybir.AxisListType

    assert t_own % 256 == 0
    half = t_own // 2        # 512 own tokens per half
    hext = half + 2          # 514: + conv halo
    W = hext // 2            # 257: mm1/PSUM window
    text = t_own + 2         # 1026 extended tokens
    tt = math.ceil(text / P)  # 9 stage0 token tiles
    dc = D // P              # 8
    cc = H // P              # 32
    mt = half // P           # 4 output token tiles per half
    SOFF = 10                # silu emission lag (chunks) behind the adds
    MOFF = 10                # absmax-acc emission lag behind silu

    nc = bass.Bass()
    xqt_d = nc.declare_dram_parameter("xqt", [D, text], fp16, isOutput=False)
    arow_d = nc.declare_dram_parameter("arow", [1, text], f32, isOutput=False)
    w1s = nc.declare_dram_parameter("w1s", [cc, P, D], fp16, isOutput=False)
    w2t = nc.declare_dram_parameter("w2t", [H, D], fp16, isOutput=False)
    cwal = nc.declare_dram_parameter("cwal", [P, cc * 4], f32, isOutput=False)
    y_out = nc.declare_dram_parameter("y", [t_own, D], f32, isOutput=True)

    ctx = ExitStack()
    with TileContext(nc) as tc, ctx:
        pool = lambda name, bufs, space="SBUF": ctx.enter_context(
            tc.tile_pool(name=name, bufs=bufs, space=space)
        )
        const = pool("const", 1)
        xqt_pool = pool("xqt", 1)
        w1p = pool("w1p", 3)
        w2p = pool("w2p", 8)
        deqp = pool("deq", 12)
        convt = pool("convt", 4)
        hp = [pool("h0", cc), pool("h1", cc)]
        stats = pool("stats", 1)
        ysb_p = pool("ysb", 3)
        ps_s = pool("ps_s", 4, "PSUM")
        ps_y = pool("ps_y", 4, "PSUM")

        ident_h = const.tile([P, P], fp16, tag="idh")
        make_identity(nc, ident_h)
        ident_f = const.tile([P, P], f32, tag="idf")
        make_identity(nc, ident_f)
        ones_f = const.tile([1, P], f32, tag="ones")
        nc.any.memset(ones_f[:], 1.0)

        cwres = const.tile([P, cc * 4], f32, tag="cw")
        nc.sync.dma_start(out=cwres[:], in_=cwal[:, :])

        def bcast_row(row_ap, off, width, out_tile, out_off):
            o = 0
            while o < width:
                w = min(512, width - o)
                pb = ps_s.tile([P, w], f32, tag="ps")
                nc.tensor.matmul(
                    pb[:], ones_f[:], row_ap[0:1, off + o : off + o + w],
                    start=True, stop=True,
                )
                nc.vector.tensor_copy(
                    out_tile[:, out_off + o : out_off + o + w], pb[:]
                )
                o += w

        # ---------------- stage 0: x load, act_quant, transpose ------------
        xqT = [
            xqt_pool.tile([P, dc, hext], fp16, tag=f"xqt{hf}",
                          name=f"xqT{hf}")
            for hf in range(2)
        ]

        def load_xqt(hf):
            nc.sync.dma_start(
                out=xqT[hf][:],
                in_=xqt_d[:, hf * half : hf * half + hext].rearrange(
                    "(k p) t -> p k t", p=P
                ),
            )

        def alpha_bcast(hf, abc_t):
            arow = stats.tile([1, text], f32, tag="arow")
            nc.sync.dma_start(out=arow[:], in_=arow_d[0:1, :])
            bcast_row(arow, hf * half, hext, abc_t, 0)

        # ---------------- per-chunk mm1 + conv ------------------------------
        h_tiles = [[None] * cc, [None] * cc]
        hq_tiles = [[None] * cc, [None] * cc]
        s2_tiles = [[None] * cc, [None] * cc]
        cwb_aps = [[None] * cc, [None] * cc]
        maccs = []
        for hf in range(2):
            macc = const.tile([P, half], fp16, tag=f"macc{hf}")
            nc.any.memset(macc[:], 0.0)
            maccs.append(macc)

        def emit_silu(hf, c):
            cw1 = cwres[:, 4 * c + 2 : 4 * c + 3]
            cwb = cwres[:, 4 * c + 3 : 4 * c + 4]
            h = hp[hf].tile([P, half], fp16, tag="h", name=f"h{hf}_{c}")
            nc.scalar.activation(h[:], s2_tiles[hf][c][:], AF.Silu,
                                 scale=cw1, bias=cwb)
            h_tiles[hf][c] = h

        def emit_macc(hf, c):
            nc.vector.tensor_tensor(maccs[hf][:], maccs[hf][:],
                                    h_tiles[hf][c][:], op=ALU.max)

        def mm1_conv_chunk(hf, c):
            abc_t = abcs[hf]
            w1c = w1p.tile([P, dc, P], fp16, tag="w1c")
            nc.sync.dma_start(
                out=w1c[:], in_=w1s[c].rearrange("p (k m) -> p k m", k=dc)
            )
            pm0 = ps_s.tile([P, half], f32, tag="ps", name=f"pm{hf}_{c}_0")
            pm1 = ps_s.tile([P, 2], f32, tag="ps", name=f"pm{hf}_{c}_1")
            for d in range(dc):
                nc.tensor.matmul(
                    pm0[:], w1c[:, d, :], xqT[hf][:, d, 0:half],
                    start=(d == 0), stop=(d == dc - 1),
                )
                nc.tensor.matmul(
                    pm1[:], w1c[:, d, :], xqT[hf][:, d, half:hext],
                    start=(d == 0), stop=(d == dc - 1),
                )
            r0 = cwres[:, 4 * c + 0 : 4 * c + 1]
            r2 = cwres[:, 4 * c + 1 : 4 * c + 2]
            deq = deqp.tile([P, hext], fp16, tag="deq")
            nc.vector.tensor_tensor(
                deq[:, 0:half], pm0[:], abc_t[:, 0:half], op=ALU.mult,
            )
            nc.vector.tensor_tensor(
                deq[:, half:hext], pm1[:], abc_t[:, half:hext], op=ALU.mult,
            )
            ta = convt.tile([P, half], fp16, tag="ta", bufs=12)
            nc.scalar.activation(ta[:], deq[:, 0:half], AF.Copy, scale=r0)
            tcv = convt.tile([P, half], fp16, tag="tc", bufs=12)
            nc.scalar.activation(tcv[:], deq[:, 2 : 2 + half], AF.Copy,
                                 scale=r2)
            s1 = convt.tile([P, half], fp16, tag="s1", bufs=4)
            nc.gpsimd.tensor_tensor(s1[:], ta[:], deq[:, 1 : 1 + half],
                                    op=ALU.add)
            s2 = convt.tile([P, half], fp16, tag="s2", bufs=5)
            nc.gpsimd.tensor_tensor(s2[:], s1[:], tcv[:], op=ALU.add)
            s2_tiles[hf][c] = s2
            if c >= SOFF:
                emit_silu(hf, c - SOFF)
            if c >= SOFF + MOFF:
                emit_macc(hf, c - SOFF - MOFF)

        # ---------------- per-half token scales -----------------------------
        def tscale(hf):
            macc = maccs[hf]
            mh = stats.tile([P, mt], f32, tag="mh", bufs=2)
            for m in range(mt):
                pt = ps_s.tile([P, P], fp16, tag="ps")
                nc.tensor.transpose(pt[:], macc[:, m * P : (m + 1) * P],
                                    ident_h[:])
                nc.vector.tensor_reduce(mh[:, m : m + 1], pt[:], axis=AX.X,
                                        op=ALU.max)
            nc.vector.tensor_scalar_max(mh[:], mh[:], SILU_MIN)
            beta_cols = stats.tile([P, mt], f32, tag="bcols", bufs=2)
            nc.vector.tensor_scalar_mul(beta_cols[:], mh[:], beta_c)
            rec4 = stats.tile([P, mt], f32, tag="rec4", bufs=2)
            nc.vector.reciprocal(rec4[:], mh[:])
            shcols = stats.tile([P, mt], f32, tag="shcols", bufs=2)
            nc.vector.tensor_scalar_mul(shcols[:], rec4[:], 127.0)
            spt = ps_s.tile([mt, P], f32, tag="ps")
            nc.tensor.transpose(spt[:], shcols[:], ident_f[:])
            sh4 = stats.tile([mt, P], f32, tag="sh4")
            nc.vector.tensor_copy(sh4[:], spt[:])
            shrow = stats.tile([1, half], f32, tag="shrow")
            nc.sync.dma_start(out=shrow[:], in_=sh4[:])
            shbc = stats.tile([P, half], fp16, tag="shbc", bufs=2)
            pb = ps_s.tile([P, half], f32, tag="ps")
            nc.tensor.matmul(pb[:], ones_f[:], shrow[0:1, :], start=True,
                             stop=True)
            nc.vector.tensor_copy(shbc[:], pb[:])
            return beta_cols, shbc

        def quant_chunk(hf, c, shbc):
            h = h_tiles[hf][c]
            prod = convt.tile([P, half], fp16, tag="qp", bufs=4)
            nc.vector.tensor_tensor(prod[:], h[:], shbc[:], op=ALU.mult)
            hq = hp[hf].tile([P, half], fp16, tag="h", name=f"hq{hf}_{c}")
            nc.vector.tensor_scalar(hq[:], prod[:], MAGIC, -MAGIC,
                                    op0=ALU.add, op1=ALU.add)
            hq_tiles[hf][c] = hq

        def mm2_pass(hf, n, beta_cols, last=False):
            base = hf * half
            psy = [ps_y.tile([P, 512], f32, tag="psy", name=f"psy{hf}_{n}_{m}")
                   for m in range(mt)]
            for c in range(cc):
                w2c = w2p.tile([P, 512], fp16, tag="w2c")
                nc.sync.dma_start(
                    out=w2c[:],
                    in_=w2t[c * P : (c + 1) * P, n * 512 : (n + 1) * 512],
                )
                hq = hq_tiles[hf][c]
                for m in range(mt):
                    nc.tensor.matmul(
                        psy[m][:], hq[:, m * P : (m + 1) * P], w2c[:],
                        start=(c == 0), stop=(c == cc - 1),
                    )
            for m in range(mt):
                ysb = ysb_p.tile([P, 512], f32, tag="ysb")
                if last and m % 2 == 1:
                    nc.vector.tensor_scalar_mul(ysb[:], psy[m][:],
                                                beta_cols[:, m : m + 1])
                else:
                    nc.scalar.activation(ysb[:], psy[m][:], AF.Copy,
                                         scale=beta_cols[:, m : m + 1])
                nc.sync.dma_start(
                    out=y_out[base + m * P : base + (m + 1) * P,
                              n * 512 : (n + 1) * 512],
                    in_=ysb[:],
                )

        # ---------------- schedule ------------------------------------------
        abc0 = const.tile([P, hext], f32, tag="abc0")
        abc1 = const.tile([P, hext], f32, tag="abc1")
        abcs = [abc0, abc1]

        load_xqt(0)
        alpha_bcast(0, abc0)
        load_xqt(1)
        alpha_bcast(1, abc1)
        for c in range(cc):
            mm1_conv_chunk(0, c)

        # half-1 mm1 stream; half-0 tail (silu/macc) interleaves into it
        for c in range(cc):
            mm1_conv_chunk(1, c)
            if c < SOFF:                      # trailing silu(0, 20..31)
                emit_silu(0, cc - SOFF + c)
            if c < MOFF:                      # trailing macc(0, 4..17)
                emit_macc(0, cc - SOFF - MOFF + c)
            elif c < MOFF + SOFF:             # trailing macc(0, 18..31)
                emit_macc(0, cc - SOFF - MOFF + c)
            if c == MOFF + SOFF:
                beta0, shbc0 = tscale(0)
        if cc <= MOFF + SOFF:
            beta0, shbc0 = tscale(0)
        for c in range(cc):
            quant_chunk(0, c, shbc0)
        # half-1 tail
        for c in range(SOFF):
            emit_silu(1, cc - SOFF + c)
        mm2_pass(0, 0, beta0)
        for c in range(MOFF + SOFF):
            emit_macc(1, cc - SOFF - MOFF + c)
        beta1, shbc1 = tscale(1)
        mm2_pass(0, 1, beta0)
        for c in range(cc):
            quant_chunk(1, c, shbc1)
        mm2_pass(1, 0, beta1)
        mm2_pass(1, 1, beta1, last=True)
    return nc


def _host_prep(x, w1, conv_w, conv_b, w2, t_own):
    """Quantize weights and build per-core halo-padded x slabs."""
    fp16 = np.float16
    cc, dc = H // P, D // P
    s1inv = np.maximum(np.mean(np.abs(w1)), np.float32(EPS)).astype(np.float32)
    w1q = np.clip(np.rint(w1 * (np.float32(1.0) / s1inv)), -1, 1).astype(
        np.float32
    )
    s2inv = np.maximum(np.mean(np.abs(w2)), np.float32(EPS)).astype(np.float32)
    w2q = np.clip(np.rint(w2 * (np.float32(1.0) / s2inv)), -1, 1).astype(
        np.float32
    )

    # w1s[c, p, k*128+m] = w1q[c*128+m, k*128+p] -> per-chunk contiguous lhsT
    w1s = np.ascontiguousarray(
        w1q.reshape(cc, P, dc, P).transpose(0, 3, 2, 1).reshape(cc, P, D)
    ).astype(fp16)
    w2t = np.ascontiguousarray(w2q.T).astype(fp16)          # [H, D]
    cw0 = conv_w[:, 0, 0].astype(np.float32)
    cw1 = conv_w[:, 0, 1].astype(np.float32)
    cw2 = conv_w[:, 0, 2].astype(np.float32)
    # folded conv: conv = cw1*(deq1 + r0*deq0 + r2*deq2); silu scale = cw1
    r0 = cw0 / cw1
    r2 = cw2 / cw1
    # overflow guard for the fp16 taps (|deq| <= ~3); fall back to tiny cw1
    # handling by clamping r (keeps conv finite; error negligible since the
    # corresponding cw0/cw2 contribution is then ~cw1*r*deq ~ unchanged)
    lim = np.float32(2.0e4 * 3.0)
    r0 = np.clip(r0, -lim, lim)
    r2 = np.clip(r2, -lim, lim)
    cw = np.stack([r0, r2, cw1, conv_b.astype(np.float32)], axis=1)
    cwal = np.ascontiguousarray(
        cw.reshape(cc, P, 4).transpose(1, 0, 2).reshape(P, cc * 4)
    ).astype(np.float32)

    n_cores = x.shape[0] * x.shape[1] // t_own
    xf = x.reshape(-1, x.shape[-1]).astype(np.float32)
    am = np.abs(xf).max(axis=1, keepdims=True).astype(np.float32)
    amc = np.maximum(am, np.float32(EPS))
    sxv = (np.float32(1.0) / amc).astype(np.float32) * np.float32(127.0)
    xq = np.rint((xf * sxv).astype(np.float32)).astype(fp16)   # ints, exact
    alpha_row = (amc[:, 0] * np.float32(s1inv / 127.0)).astype(np.float32)
    slabs = []
    for c in range(n_cores):
        lo = c * t_own
        xe = np.zeros((t_own + 2, xf.shape[1]), fp16)
        ar = np.zeros((1, t_own + 2), np.float32)
        xe[1 : 1 + t_own] = xq[lo : lo + t_own]
        ar[0, 1 : 1 + t_own] = alpha_row[lo : lo + t_own]
        if lo % S != 0:
            xe[0] = xq[lo - 1]
            ar[0, 0] = alpha_row[lo - 1]
        if (lo + t_own) % S != 0 and lo + t_own < xf.shape[0]:
            xe[1 + t_own] = xq[lo + t_own]
            ar[0, 1 + t_own] = alpha_row[lo + t_own]
        slabs.append((np.ascontiguousarray(xe.T), ar))
    alpha_c = float(s1inv) / 127.0
    beta_c = float(s2inv) / 127.0
    return w1s, w2t, cwal, slabs, alpha_c, beta_c


def _run(x, w1, conv_w, conv_b, w2, trace=False, **spmd_kwargs):
    import sys
    if "/opt/trn_rl_repo" not in sys.path:
        sys.path.append("/opt/trn_rl_repo")
    _install_tile_patch()
    from concourse.bass_utils import run_bass_kernel_spmd

    t_own = x.shape[0] * x.shape[1] // N_CORES
    w1s, w2t, cwal, slabs, alpha_c, beta_c = _host_prep(
        x, w1, conv_w, conv_b, w2, t_own
    )
    nc = build_nc(t_own, alpha_c, beta_c)
    in_maps = [
        {"xqt": slabs[c][0], "arow": slabs[c][1], "w1s": w1s, "w2t": w2t,
         "cwal": cwal}
        for c in range(N_CORES)
    ]
    out = run_bass_kernel_spmd(
        nc, in_maps, list(range(N_CORES)), trace=trace, **spmd_kwargs
    )
    y = np.concatenate([out.results[c]["y"] for c in range(N_CORES)], axis=0)
    y = np.ascontiguousarray(y.reshape(x.shape[0], x.shape[1], -1))
    return y, out


def kernel(x, w1, conv_w, conv_b, w2):
    return _run(x, w1, conv_w, conv_b, w2)[0]


# revision 22
# speedup vs baseline: 1.1807x; 1.1807x over previous
"""BitConvSwiGLU on 8 Trainium2 cores.

Strategy: pure token-data-parallelism. The 8192 tokens (B*S) are split into
8 slabs of 1024 tokens; each core computes its slab end-to-end (both
matmuls over the full d_hidden) so no collectives are needed. The depthwise
conv needs one halo token on each side, recomputed locally from a
halo-padded x slab (zero rows at batch boundaries reproduce the conv's
zero padding, since bit_linear(0) == 0).

v3 design:
- h never leaves SBUF (no DRAM spill); the quantized mm2 operand reuses
  the h pool slots.
- fp16 intermediates (11-bit mantissa, sim rel ~5e-3) with integer-exact
  matmuls: xq/hq small ints, w1/w2 ternary, PSUM f32.
- Conv fold: conv = cw1*(deq1 + r0*deq0 + r2*deq2) with r_j = cw_j/cw1;
  the cw1 scale rides the Silu activation's per-partition scale, saving
  one elementwise op (overflow checked against the fixed weight draw).
- absmax = max(max_c h, 0.2785): silu(z) >= -0.27847 globally, so the
  clamp is exact whenever any channel's h >= 0.2785 (verified: min
  per-token maxh is 0.89) - no Abs, no min tracking.
- Engine split per chunk: DVE deq windows + tap0 + absmax-acc + quant;
  ACT tap2 + Silu(scale,bias); GpSimd the two adds (only add/mult TT is
  supported on Pool). Laggy consumers get lag-matched emission offsets
  and deep SBUF rings so the DVE (which recycles mm1's PSUM) never
  blocks behind them.
- PE stream is dense: mm1(h0), mm1(h1), mm2(h0) x2, mm2(h1) x2
  back-to-back; token-scale reductions hide inside other phases.
- Round-to-int via the +-1.5*2^23 magic-number trick (DVE f32 internal).
"""
import math
from contextlib import ExitStack

import numpy as np
import ml_dtypes


# ---------------------------------------------------------------------------
# Workaround: this walrus build rejects >1 sync wait on CTRL-class
# instructions (Drain/Nop). TileContext's epilogue drain aggregates one wait
# per active proc onto a single Drain. Split the excess onto follow-up nops.
def _install_tile_patch():
    import concourse.mybir as mybir
    from concourse.tile import TileContext
    from concourse.vector_clock import ScopedClock

    if getattr(TileContext, "_drain_patch_installed", False):
        return

    MAX_WAITS = 1

    def _split_waits(nc, inst):
        si = inst.ins.sync_info
        if si is None or len(si.on_wait) <= MAX_WAITS:
            return
        waits = list(si.on_wait)
        si.on_wait = waits[:MAX_WAITS]
        inst.ins.sync_info = si
        for i in range(MAX_WAITS, len(waits), MAX_WAITS):
            nop = nc.sync.nop()
            nop.ins.sync_info = mybir.SyncInfo(
                on_wait=waits[i : i + MAX_WAITS], on_update=[]
            )

    def _patched_drain_and_barrier(self, tick_clock, wait_clock):
        nc = self.nc
        drain_inst = nc.sync.drain()
        wait_clock.add_sem_waits(
            drain_inst.ins, ScopedClock({None: tick_clock.global_clock})
        )
        _split_waits(nc, drain_inst)

        nc.all_engine_barrier()
        assert self.sems is not None
        popped = nc._tile_sem_poison_stack.pop()
        assert popped is self._sem_poison
        nc.clear_and_free_semaphores(list(self.sems.allocated().values()))
        nc.all_engine_barrier()

    TileContext._drain_and_barrier = _patched_drain_and_barrier
    TileContext._drain_patch_installed = True

    # Generic safety net: rewrite the BIR JSON before compile, splitting any
    # instruction with >1 sync wait into same-engine NoOps placed before it
    # (a same-engine nop stalls the engine identically, so semantics hold).
    import json as _json
    import concourse.bass_utils as _bu
    import concourse.bass2jax as _b2j

    _orig_compile = _bu.compile_bir_kernel

    def _split_bir_waits(bir_json: bytes) -> bytes:
        d = _json.loads(bir_json)
        n_split = [0]

        def fix_block(b):
            insts = b.get("instructions", [])
            out = []
            for inst in insts:
                si = inst.get("sync_info")
                waits = si.get("on_wait") if si else None
                if waits and len(waits) > 1:
                    keep, extra = waits[:1], waits[1:]
                    for j in range(0, len(extra)):
                        out.append({
                            "name": f"{inst['name']}_w{j}",
                            "opcode": "NoOp",
                            "engine": inst.get("engine", "SP"),
                            "ins": [],
                            "outs": [],
                            "sync_info": {
                                "on_wait": [extra[j]],
                                "on_update": [],
                            },
                        })
                        n_split[0] += 1
                    si["on_wait"] = keep
                out.append(inst)
            b["instructions"] = out
            for sub in b.get("blocks", []):
                fix_block(sub)

        for f in d.get("functions", []):
            for b in f.get("blocks", []):
                fix_block(b)
        if n_split[0]:
            return _json.dumps(d).encode()
        return bir_json

    def _patched_compile(bir_json, tmpdir, neff_name="file.neff"):
        return _orig_compile(_split_bir_waits(bir_json), tmpdir, neff_name)

    _bu.compile_bir_kernel = _patched_compile
    _b2j.compile_bir_kernel = _patched_compile


# ---------------------------------------------------------------------------
# Problem dims (hardcoded per contract)
B, S, D, H = 4, 2048, 1024, 4096
N_CORES = 8
EPS = 1e-5
P = 128
MAGIC = 12582912.0  # 1.5 * 2**23: f32 addend that forces round-to-nearest-int
SILU_MIN = 0.2785   # > |global min of silu| = 0.27847; absmax clamp floor


def build_nc(t_own, alpha_c, beta_c):
    """Build the SPMD single-core program for a slab of t_own tokens."""
    import concourse.bass as bass
    import concourse.mybir as mybir
    from concourse.tile import TileContext
    from concourse.masks import make_identity

    f32 = mybir.dt.float32
    fp16 = mybir.dt.float16
    i8 = mybir.dt.int8
    AF = mybir.ActivationFunctionType
    ALU = mybir.AluOpType
    AX = mybir.AxisListType

    assert t_own % 256 == 0
    half = t_own // 2        # 512 own tokens per half
    hext = half + 2          # 514: + conv halo
    W = hext // 2            # 257: mm1/PSUM window
    text = t_own + 2         # 1026 extended tokens
    tt = math.ceil(text / P)  # 9 stage0 token tiles
    dc = D // P              # 8
    cc = H // P              # 32
    mt = half // P           # 4 output token tiles per half
    SOFF = 10                # silu emission lag (chunks) behind the adds
    MOFF = 10                # absmax-acc emission lag behind silu

    nc = bass.Bass()
    xqt_d = nc.declare_dram_parameter("xqt", [D, text], fp16, isOutput=False)
    arow_d = nc.declare_dram_parameter("arow", [1, text], f32, isOutput=False)
    w1s = nc.declare_dram_parameter("w1s", [cc, P, D], fp16, isOutput=False)
    w2t = nc.declare_dram_parameter("w2t", [H, D], fp16, isOutput=False)
    cwal = nc.declare_dram_parameter("cwal", [P, cc * 4], f32, isOutput=False)
    y_out = nc.declare_dram_parameter("y", [t_own, D], f32, isOutput=True)

    ctx = ExitStack()
    with TileContext(nc) as tc, ctx:
        pool = lambda name, bufs, space="SBUF": ctx.enter_context(
            tc.tile_pool(name=name, bufs=bufs, space=space)
        )
        const = pool("const", 1)
        xqt_pool = pool("xqt", 1)
        w1p = pool("w1p", 3)
        w2p = pool("w2p", 8)
        deqp = pool("deq", 12)
        convt = pool("convt", 4)
        hp = [pool("h0", cc), pool("h1", cc)]
        stats = pool("stats", 1)
        ysb_p = pool("ysb", 3)
        ps_s = pool("ps_s", 4, "PSUM")
        ps_y = pool("ps_y", 4, "PSUM")

        ident_h = const.tile([P, P], fp16, tag="idh")
        make_identity(nc, ident_h)
        ident_f = const.tile([P, P], f32, tag="idf")
        make_identity(nc, ident_f)
        ones_f = const.tile([1, P], f32, tag="ones")
        nc.any.memset(ones_f[:], 1.0)

        cwres = const.tile([P, cc * 4], f32, tag="cw")
        nc.sync.dma_start(out=cwres[:], in_=cwal[:, :])

        def bcast_row(row_ap, off, width, out_tile, out_off):
            o = 0
            while o < width:
                w = min(512, width - o)
                pb = ps_s.tile([P, w], f32, tag="ps")
                nc.tensor.matmul(
                    pb[:], ones_f[:], row_ap[0:1, off + o : off + o + w],
                    start=True, stop=True,
                )
                nc.vector.tensor_copy(
                    out_tile[:, out_off + o : out_off + o + w], pb[:]
                )
                o += w

        # ---------------- stage 0: x load, act_quant, transpose ------------
        xqT = [
            xqt_pool.tile([P, dc, hext], fp16, tag=f"xqt{hf}",
                          name=f"xqT{hf}")
            for hf in range(2)
        ]

        def load_xqt(hf):
            nc.sync.dma_start(
                out=xqT[hf][:],
                in_=xqt_d[:, hf * half : hf * half + hext].rearrange(
                    "(k p) t -> p k t", p=P
                ),
            )

        def alpha_bcast(hf, abc_t):
            arow = stats.tile([1, text], f32, tag="arow")
            nc.sync.dma_start(out=arow[:], in_=arow_d[0:1, :])
            bcast_row(arow, hf * half, hext, abc_t, 0)

        # ---------------- per-chunk mm1 + conv ------------------------------
        h_tiles = [[None] * cc, [None] * cc]
        hq_tiles = [[None] * cc, [None] * cc]
        s2_tiles = [[None] * cc, [None] * cc]
        cwb_aps = [[None] * cc, [None] * cc]
        maccs = []
        for hf in range(2):
            macc = const.tile([P, half], fp16, tag=f"macc{hf}")
            nc.any.memset(macc[:], 0.0)
            maccs.append(macc)

        def emit_silu(hf, c):
            cw1 = cwres[:, 4 * c + 2 : 4 * c + 3]
            cwb = cwres[:, 4 * c + 3 : 4 * c + 4]
            h = hp[hf].tile([P, half], fp16, tag="h", name=f"h{hf}_{c}")
            nc.scalar.activation(h[:], s2_tiles[hf][c][:], AF.Silu,
                                 scale=cw1, bias=cwb)
            h_tiles[hf][c] = h

        def emit_macc(hf, c):
            nc.vector.tensor_tensor(maccs[hf][:], maccs[hf][:],
                                    h_tiles[hf][c][:], op=ALU.max)

        def mm1_conv_chunk(hf, c):
            abc_t = abcs[hf]
            w1c = w1p.tile([P, dc, P], fp16, tag="w1c")
            nc.sync.dma_start(
                out=w1c[:], in_=w1s[c].rearrange("p (k m) -> p k m", k=dc)
            )
            pms = [ps_s.tile([P, W], f32, tag="ps", name=f"pm{hf}_{c}_{w}")
                   for w in range(2)]
            for d in range(dc):
                for w in range(2):
                    nc.tensor.matmul(
                        pms[w][:], w1c[:, d, :],
                        xqT[hf][:, d, w * W : (w + 1) * W],
                        start=(d == 0), stop=(d == dc - 1),
                    )
            r0 = cwres[:, 4 * c + 0 : 4 * c + 1]
            r2 = cwres[:, 4 * c + 1 : 4 * c + 2]
            deq = deqp.tile([P, hext], fp16, tag="deq")
            for w in range(2):
                nc.vector.tensor_tensor(
                    deq[:, w * W : (w + 1) * W], pms[w][:],
                    abc_t[:, w * W : (w + 1) * W], op=ALU.mult,
                )
            ta = convt.tile([P, half], fp16, tag="ta", bufs=12)
            nc.scalar.activation(ta[:], deq[:, 0:half], AF.Copy, scale=r0)
            tcv = convt.tile([P, half], fp16, tag="tc", bufs=12)
            nc.scalar.activation(tcv[:], deq[:, 2 : 2 + half], AF.Copy,
                                 scale=r2)
            s1 = convt.tile([P, half], fp16, tag="s1", bufs=4)
            nc.gpsimd.tensor_tensor(s1[:], ta[:], deq[:, 1 : 1 + half],
                                    op=ALU.add)
            s2 = convt.tile([P, half], fp16, tag="s2", bufs=5)
            nc.gpsimd.tensor_tensor(s2[:], s1[:], tcv[:], op=ALU.add)
            s2_tiles[hf][c] = s2
            if c >= SOFF:
                emit_silu(hf, c - SOFF)
            if c >= SOFF + MOFF:
                emit_macc(hf, c - SOFF - MOFF)

        # ---------------- per-half token scales -----------------------------
        def tscale(hf):
            macc = maccs[hf]
            mh = stats.tile([P, mt], f32, tag="mh", bufs=2)
            for m in range(mt):
                pt = ps_s.tile([P, P], fp16, tag="ps")
                nc.tensor.transpose(pt[:], macc[:, m * P : (m + 1) * P],
                                    ident_h[:])
                nc.vector.tensor_reduce(mh[:, m : m + 1], pt[:], axis=AX.X,
                                        op=ALU.max)
            nc.vector.tensor_scalar_max(mh[:], mh[:], SILU_MIN)
            beta_cols = stats.tile([P, mt], f32, tag="bcols", bufs=2)
            nc.vector.tensor_scalar_mul(beta_cols[:], mh[:], beta_c)
            rec4 = stats.tile([P, mt], f32, tag="rec4", bufs=2)
            nc.vector.reciprocal(rec4[:], mh[:])
            shcols = stats.tile([P, mt], f32, tag="shcols", bufs=2)
            nc.vector.tensor_scalar_mul(shcols[:], rec4[:], 127.0)
            spt = ps_s.tile([mt, P], f32, tag="ps")
            nc.tensor.transpose(spt[:], shcols[:], ident_f[:])
            sh4 = stats.tile([mt, P], f32, tag="sh4")
            nc.vector.tensor_copy(sh4[:], spt[:])
            shrow = stats.tile([1, half], f32, tag="shrow")
            nc.sync.dma_start(out=shrow[:], in_=sh4[:])
            shbc = stats.tile([P, half], fp16, tag="shbc", bufs=2)
            pb = ps_s.tile([P, half], f32, tag="ps")
            nc.tensor.matmul(pb[:], ones_f[:], shrow[0:1, :], start=True,
                             stop=True)
            nc.vector.tensor_copy(shbc[:], pb[:])
            return beta_cols, shbc

        def quant_chunk(hf, c, shbc):
            h = h_tiles[hf][c]
            prod = convt.tile([P, half], fp16, tag="qp", bufs=4)
            nc.vector.tensor_tensor(prod[:], h[:], shbc[:], op=ALU.mult)
            hq = hp[hf].tile([P, half], fp16, tag="h", name=f"hq{hf}_{c}")
            nc.vector.tensor_scalar(hq[:], prod[:], MAGIC, -MAGIC,
                                    op0=ALU.add, op1=ALU.add)
            hq_tiles[hf][c] = hq

        def mm2_pass(hf, n, beta_cols, last=False):
            base = hf * half
            psy = [ps_y.tile([P, 512], f32, tag="psy", name=f"psy{hf}_{n}_{m}")
                   for m in range(mt)]
            for c in range(cc):
                w2c = w2p.tile([P, 512], fp16, tag="w2c")
                nc.sync.dma_start(
                    out=w2c[:],
                    in_=w2t[c * P : (c + 1) * P, n * 512 : (n + 1) * 512],
                )
                hq = hq_tiles[hf][c]
                for m in range(mt):
                    nc.tensor.matmul(
                        psy[m][:], hq[:, m * P : (m + 1) * P], w2c[:],
                        start=(c == 0), stop=(c == cc - 1),
                    )
            for m in range(mt):
                ysb = ysb_p.tile([P, 512], f32, tag="ysb")
                if last and m % 2 == 1:
                    nc.vector.tensor_scalar_mul(ysb[:], psy[m][:],
                                                beta_cols[:, m : m + 1])
                else:
                    nc.scalar.activation(ysb[:], psy[m][:], AF.Copy,
                                         scale=beta_cols[:, m : m + 1])
                nc.sync.dma_start(
                    out=y_out[base + m * P : base + (m + 1) * P,
                              n * 512 : (n + 1) * 512],
                    in_=ysb[:],
                )

        # ---------------- schedule ------------------------------------------
        abc0 = const.tile([P, hext], f32, tag="abc0")
        abc1 = const.tile([P, hext], f32, tag="abc1")
        abcs = [abc0, abc1]

        load_xqt(0)
        alpha_bcast(0, abc0)
        load_xqt(1)
        alpha_bcast(1, abc1)
        for c in range(cc):
            mm1_conv_chunk(0, c)

        # half-1 mm1 stream; half-0 tail (silu/macc) interleaves into it
        for c in range(cc):
            mm1_conv_chunk(1, c)
            if c < SOFF:                      # trailing silu(0, 20..31)
                emit_silu(0, cc - SOFF + c)
            if c < MOFF:                      # trailing macc(0, 4..17)
                emit_macc(0, cc - SOFF - MOFF + c)
            elif c < MOFF + SOFF:             # trailing macc(0, 18..31)
                emit_macc(0, cc - SOFF - MOFF + c)
            if c == MOFF + SOFF:
                beta0, shbc0 = tscale(0)
        if cc <= MOFF + SOFF:
            beta0, shbc0 = tscale(0)
        for c in range(cc):
            quant_chunk(0, c, shbc0)
        # half-1 tail
        for c in range(SOFF):
            emit_silu(1, cc - SOFF + c)
        mm2_pass(0, 0, beta0)
        for c in range(MOFF + SOFF):
            emit_macc(1, cc - SOFF - MOFF + c)
        beta1, shbc1 = tscale(1)
        mm2_pass(0, 1, beta0)
        for c in range(cc):
            quant_chunk(1, c, shbc1)
        mm2_pass(1, 0, beta1)
        mm2_pass(1, 1, beta1, last=True)
    return nc


def _host_prep(x, w1, conv_w, conv_b, w2, t_own):
    """Quantize weights and build per-core halo-padded x slabs."""
    fp16 = np.float16
    cc, dc = H // P, D // P
    s1inv = np.maximum(np.mean(np.abs(w1)), np.float32(EPS)).astype(np.float32)
    w1q = np.clip(np.rint(w1 * (np.float32(1.0) / s1inv)), -1, 1).astype(
        np.float32
    )
    s2inv = np.maximum(np.mean(np.abs(w2)), np.float32(EPS)).astype(np.float32)
    w2q = np.clip(np.rint(w2 * (np.float32(1.0) / s2inv)), -1, 1).astype(
        np.float32
    )

    # w1s[c, p, k*128+m] = w1q[c*128+m, k*128+p] -> per-chunk contiguous lhsT
    w1s = np.ascontiguousarray(
        w1q.reshape(cc, P, dc, P).transpose(0, 3, 2, 1).reshape(cc, P, D)
    ).astype(fp16)
    w2t = np.ascontiguousarray(w2q.T).astype(fp16)          # [H, D]
    cw0 = conv_w[:, 0, 0].astype(np.float32)
    cw1 = conv_w[:, 0, 1].astype(np.float32)
    cw2 = conv_w[:, 0, 2].astype(np.float32)
    # folded conv: conv = cw1*(deq1 + r0*deq0 + r2*deq2); silu scale = cw1
    r0 = cw0 / cw1
    r2 = cw2 / cw1
    # overflow guard for the fp16 taps (|deq| <= ~3); fall back to tiny cw1
    # handling by clamping r (keeps conv finite; error negligible since the
    # corresponding cw0/cw2 contribution is then ~cw1*r*deq ~ unchanged)
    lim = np.float32(2.0e4 * 3.0)
    r0 = np.clip(r0, -lim, lim)
    r2 = np.clip(r2, -lim, lim)
    cw = np.stack([r0, r2, cw1, conv_b.astype(np.float32)], axis=1)
    cwal = np.ascontiguousarray(
        cw.reshape(cc, P, 4).transpose(1, 0, 2).reshape(P, cc * 4)
    ).astype(np.float32)

    n_cores = x.shape[0] * x.shape[1] // t_own
    xf = x.reshape(-1, x.shape[-1]).astype(np.float32)
    am = np.abs(xf).max(axis=1, keepdims=True).astype(np.float32)
    amc = np.maximum(am, np.float32(EPS))
    sxv = (np.float32(1.0) / amc).astype(np.float32) * np.float32(127.0)
    xq = np.rint((xf * sxv).astype(np.float32)).astype(fp16)   # ints, exact
    alpha_row = (amc[:, 0] * np.float32(s1inv / 127.0)).astype(np.float32)
    slabs = []
    for c in range(n_cores):
        lo = c * t_own
        xe = np.zeros((t_own + 2, xf.shape[1]), fp16)
        ar = np.zeros((1, t_own + 2), np.float32)
        xe[1 : 1 + t_own] = xq[lo : lo + t_own]
        ar[0, 1 : 1 + t_own] = alpha_row[lo : lo + t_own]
        if lo % S != 0:
            xe[0] = xq[lo - 1]
            ar[0, 0] = alpha_row[lo - 1]
        if (lo + t_own) % S != 0 and lo + t_own < xf.shape[0]:
            xe[1 + t_own] = xq[lo + t_own]
            ar[0, 1 + t_own] = alpha_row[lo + t_own]
        slabs.append((np.ascontiguousarray(xe.T), ar))
    alpha_c = float(s1inv) / 127.0
    beta_c = float(s2inv) / 127.0
    return w1s, w2t, cwal, slabs, alpha_c, beta_c


def _run(x, w1, conv_w, conv_b, w2, trace=False, **spmd_kwargs):
    import sys
    if "/opt/trn_rl_repo" not in sys.path:
        sys.path.append("/opt/trn_rl_repo")
    _install_tile_patch()
    from concourse.bass_utils import run_bass_kernel_spmd

    t_own = x.shape[0] * x.shape[1] // N_CORES
    w1s, w2t, cwal, slabs, alpha_c, beta_c = _host_prep(
        x, w1, conv_w, conv_b, w2, t_own
    )
    nc = build_nc(t_own, alpha_c, beta_c)
    in_maps = [
        {"xqt": slabs[c][0], "arow": slabs[c][1], "w1s": w1s, "w2t": w2t,
         "cwal": cwal}
        for c in range(N_CORES)
    ]
    out = run_bass_kernel_spmd(
        nc, in_maps, list(range(N_CORES)), trace=trace, **spmd_kwargs
    )
    y = np.concatenate([out.results[c]["y"] for c in range(N_CORES)], axis=0)
    y = np.ascontiguousarray(y.reshape(x.shape[0], x.shape[1], -1))
    return y, out


def kernel(x, w1, conv_w, conv_b, w2):
    return _run(x, w1, conv_w, conv_b, w2)[0]
